# revision 10
# baseline (speedup 1.0000x reference)
"""Trainium2 Bass kernel for a dense pre-LN transformer block.

Sharding: 8 cores = 4 batches x 2 sequence-halves (zigzag query blocks).
Each core handles one batch element; K/V are computed redundantly for the
full sequence on both cores of a batch (cheaper than collectives), and each
core computes attention + proj + FFN for 1024 of the 2048 query tokens.

To keep the SPMD instruction stream identical across cores, each core's
tokens are host-side permuted to [own_blockA; own_blockB; rest] and all
causal-validity variation is carried in per-core mask data (triangular
tiles for diagonal blocks, per-partition 0/1 scalars for whole chunks).

Activations/weights are bf16 (PE runs bf16 at full rate, same as fp32r);
accumulation, softmax normalizers, residuals and LN statistics stay fp32.
Transposes for hT/h2T go through the DMA xbar (dma_start_transpose), not
the PE. Weights are double-buffered across passes and prefetched.
"""

import contextlib

import numpy as np
import ml_dtypes

from concourse import bass, bacc, tile, mybir
from concourse.bass_utils import run_bass_kernel_spmd

F32 = mybir.dt.float32
F32R = mybir.dt.float32r
BF16 = mybir.dt.bfloat16

B, T, D = 4, 2048, 1024
H, HD = 16, 64
DFF = 4 * D
EPS = 1e-5
N_CORES = 8

FULL_CFG = dict(D=1024, H=16, T=2048, QB=512, DFF=4096, NG=4)
SMALL_CFG = dict(D=256, H=4, T=512, QB=128, DFF=512, NG=2)


def derive(cfg):
    c = dict(cfg)
    c["DC"] = cfg["D"] // 128            # d-chunks
    c["FC"] = cfg["H"] * HD // 128       # feature chunks (head pairs)
    c["FCP"] = 2                         # f-chunks per pass
    c["NPASS"] = c["FC"] // c["FCP"]
    c["S"] = cfg["T"] // 128             # key chunks
    c["QBC"] = cfg["QB"] // 128          # chunks per query block
    c["NT"] = cfg["QB"]                  # moving-dim tile (== query block)
    c["TOWN"] = 2 * cfg["QB"]            # tokens owned per core
    c["TOC"] = c["TOWN"] // 128
    c["NO"] = min(512, cfg["D"])
    c["OC"] = cfg["D"] // c["NO"]        # dout chunks of <=512
    c["GFC"] = (cfg["DFF"] // cfg["NG"]) // 128  # f-chunks per FFN group
    c["KTB"] = cfg["T"] // c["NT"]       # t-blocks for k over full T
    return c


def build(cfg):
    """Emit the bass program for one core. Returns nc."""
    c = derive(cfg)
    Dm, Tf, DFFm, NG = cfg["D"], cfg["T"], cfg["DFF"], cfg["NG"]
    DC, FC, FCP, NPASS = c["DC"], c["FC"], c["FCP"], c["NPASS"]
    S, QBC, NT, TOWN, TOC = c["S"], c["QBC"], c["NT"], c["TOWN"], c["TOC"]
    OC, NO, GFC, KTB = c["OC"], c["NO"], c["GFC"], c["KTB"]
    HDf = HD  # 64

    nc = bacc.Bacc("TRN2", target_bir_lowering=False, debug=False)

    # ---- DRAM I/O ----
    x_d = nc.dram_tensor("x", [Tf, Dm], BF16, kind="ExternalInput")
    xbo_d = nc.dram_tensor("xbo", [TOWN, Dm], BF16, kind="ExternalInput")
    wq_d = nc.dram_tensor("wq", [NPASS, DC, 128, FCP * 128], BF16,
                          kind="ExternalInput")
    wk_d = nc.dram_tensor("wk", [NPASS, DC, 128, FCP * 128], BF16,
                          kind="ExternalInput")
    wv_d = nc.dram_tensor("wv", [NPASS, DC, 128, FCP * 130], BF16,
                          kind="ExternalInput")
    bq_d = nc.dram_tensor("bq", [FC, 128, 1], F32, kind="ExternalInput")
    bk_d = nc.dram_tensor("bk", [FC, 128, 1], F32, kind="ExternalInput")
    bv_d = nc.dram_tensor("bv", [NPASS, 1, FCP * 130], F32,
                          kind="ExternalInput")
    wo_d = nc.dram_tensor("wo", [FC, 128, Dm], BF16, kind="ExternalInput")
    w1_d = nc.dram_tensor("w1", [NG, DC, 128, DFFm // NG], BF16,
                          kind="ExternalInput")
    b1_d = nc.dram_tensor("b1", [DFFm // 128, 128, 1], F32,
                          kind="ExternalInput")
    w2_d = nc.dram_tensor("w2", [NG, GFC, 128, Dm], BF16,
                          kind="ExternalInput")
    b2_d = nc.dram_tensor("b2", [128, Dm], F32, kind="ExternalInput")
    tri_d = nc.dram_tensor("tri", [QBC, 128, NT], BF16, kind="ExternalInput")
    cm_d = nc.dram_tensor("cm", [2 * QBC, 128, 1], F32, kind="ExternalInput")
    zro_d = nc.dram_tensor("zeros", [128, 1], F32, kind="ExternalInput")
    out_d = nc.dram_tensor("out", [TOWN, Dm], F32, kind="ExternalOutput")

    xr = x_d.ap().rearrange("(n p) d -> n p d", p=128)
    xbor = xbo_d.ap().rearrange("(n p) d -> n p d", p=128)
    outr = out_d.ap().rearrange("(n p) d -> n p d", p=128)

    with tile.TileContext(nc) as tc, contextlib.ExitStack() as top:
        cpool = top.enter_context(tc.tile_pool(name="const", bufs=1))
        cms = cpool.tile([128, 2 * QBC], F32, name="cms", tag="cms")
        nc.sync.dma_start(cms[:], cm_d.ap().rearrange("n p o -> p (n o)"))
        zbias = cpool.tile([128, 1], F32, name="zbias", tag="zbias")
        nc.sync.dma_start(zbias[:], zro_d.ap())

        # pass-weight pools (double-buffered across passes); entered before
        # the mid-kernel-released pools to keep pool LIFO order.
        wkq_pool = top.enter_context(tc.tile_pool(name="wkq", bufs=2))
        wv_pool = top.enter_context(tc.tile_pool(name="wvp", bufs=2))

        ctxp = top.enter_context(tc.tile_pool(name="ctxTp", bufs=1))
        ctxT = [ctxp.tile([128, TOWN], BF16, name=f"ctxT{fc}", tag=f"ctxT{fc}")
                for fc in range(FC)]

        hT_stack = contextlib.ExitStack()
        hp = hT_stack.enter_context(tc.tile_pool(name="hTp", bufs=1))
        hT = hp.tile([128, S, DC, 128], BF16, name="hT", tag="hT")
        trip = hT_stack.enter_context(tc.tile_pool(name="trip", bufs=1))
        tri = trip.tile([128, QBC, NT], BF16, name="tri", tag="tri")
        nc.sync.dma_start(tri[:], tri_d.ap().rearrange("q p t -> p q t"))

        # per-head biases for all passes, loaded once
        bqk = cpool.tile([128, 2 * FC], F32, name="bqk", tag="bqk")
        nc.sync.dma_start(bqk[:, :FC], bk_d.ap().rearrange("f p o -> p (f o)"))
        nc.sync.dma_start(bqk[:, FC:], bq_d.ap().rearrange("f p o -> p (f o)"))
        bvr = cpool.tile([1, NPASS, FCP * 130], F32, name="bvr", tag="bvr")
        nc.sync.dma_start(bvr[:], bv_d.ap().rearrange("n o c -> o n c"))
        bvb = cpool.tile([128, NPASS, FCP * 130], F32, name="bvb", tag="bvb")
        for pp_ in range(NPASS):
            nc.gpsimd.partition_broadcast(bvb[:, pp_, :], bvr[:, pp_, :])

        def issue_pass_loads(p):
            w = {}
            wkt = wkq_pool.tile([128, DC, FCP * 128], BF16,
                                name="wk", tag="wk")
            nc.scalar.dma_start(wkt[:], wk_d.ap()[p].rearrange(
                "dc p c -> p dc c"))
            w["wk"] = wkt
            wqt = wkq_pool.tile([128, DC, FCP * 128], BF16,
                                name="wq", tag="wq")
            nc.sync.dma_start(wqt[:], wq_d.ap()[p].rearrange(
                "dc p c -> p dc c"))
            w["wq"] = wqt
            wvt = wv_pool.tile([128, DC, FCP * 130], BF16,
                               name="wv", tag="wv")
            nc.scalar.dma_start(wvt[:], wv_d.ap()[p].rearrange(
                "dc p c -> p dc c"))
            w["wv"] = wvt
            return w

        cur_w = issue_pass_loads(0)

        # ---------------- Phase 1: LN1 -> hT (via DMA transpose) ---------
        XG = 4  # x chunks loaded per DMA
        with tc.tile_pool(name="ln1", bufs=2) as lxp, \
             tc.tile_pool(name="ln1h", bufs=4) as lp, \
             tc.tile_pool(name="ln1s", bufs=8) as lsp:
            for tig in range(S // XG):
                xt4 = lxp.tile([128, XG, Dm], BF16, name="xt4", tag="xt4")
                nc.sync.dma_start(
                    xt4[:], xr[tig * XG:(tig + 1) * XG].rearrange(
                        "n p d -> p n d"))
                for k in range(XG):
                    ti = tig * XG + k
                    xt = xt4[:, k, :]
                    nsub = max(1, Dm // 512)
                    st6 = lsp.tile([128, nsub, 6], F32, name="st6", tag="st6")
                    for sb_i in range(nsub):
                        cs = slice(sb_i * (Dm // nsub),
                                   (sb_i + 1) * (Dm // nsub))
                        nc.vector.bn_stats(st6[:, sb_i, :], xt[:, cs])
                    agg = lsp.tile([128, 2], F32, name="agg", tag="agg")
                    nc.vector.bn_aggr(agg[:], st6[:])
                    veps = lsp.tile([128, 1], F32, name="veps", tag="veps")
                    nc.vector.tensor_scalar_add(veps[:], agg[:, 1:2], EPS)
                    std = lsp.tile([128, 1], F32, name="std", tag="std")
                    nc.scalar.sqrt(std[:], veps[:])
                    rstd = lsp.tile([128, 1], F32, name="rstd", tag="rstd")
                    nc.vector.reciprocal(rstd[:], std[:])
                    ht = lp.tile([128, Dm], BF16, name="ht", tag="ht")
                    hstep = Dm // nsub
                    for sb_i in range(nsub):
                        cs = slice(sb_i * hstep, (sb_i + 1) * hstep)
                        eng = nc.vector if sb_i % 2 == 0 else nc.gpsimd
                        eng.tensor_scalar(ht[:, cs], xt[:, cs],
                                          agg[:, 0:1], rstd[:],
                                          op0=mybir.AluOpType.subtract,
                                          op1=mybir.AluOpType.mult)
                    eng = nc.sync if ti % 2 == 0 else nc.scalar
                    eng.dma_start_transpose(hT[:, ti, :, :], ht[:])

        # ---------------- Phase 2: per-pass QKV + attention -------------
        with tc.tile_pool(name="pass_sb", bufs=1) as pp, \
             tc.tile_pool(name="vtp", bufs=1) as vp, \
             tc.tile_pool(name="expp", bufs=4) as ep, \
             tc.tile_pool(name="zrowp", bufs=2) as zp, \
             tc.tile_pool(name="zbp", bufs=2) as zbp, \
             tc.tile_pool(name="qkvps", bufs=2, space=bass.MemorySpace.PSUM) as qps, \
             tc.tile_pool(name="scps", bufs=2, space=bass.MemorySpace.PSUM) as sps, \
             tc.tile_pool(name="ctxps", bufs=1, space=bass.MemorySpace.PSUM) as cps:
            for p in range(NPASS):
                fcs = [p * FCP + i for i in range(FCP)]
                w = cur_w
                # --- K^T and Q^T (feature-major) ---
                kT = [pp.tile([128, Tf], BF16, name=f"kT{i}", tag=f"kT{i}")
                      for i in range(FCP)]
                qT = [pp.tile([128, 2 * NT], BF16, name=f"qT{i}", tag=f"qT{i}")
                      for i in range(FCP)]
                for i, fc in enumerate(fcs):
                    ws = slice(i * 128, (i + 1) * 128)
                    for tb in range(KTB):
                        pk = qps.tile([128, NT], F32, name="pk", tag="qkv")
                        for dc in range(DC):
                            nc.tensor.matmul(
                                pk[:], (w["wk"][:, dc, ws]),
                                (hT[:, tb * QBC:(tb + 1) * QBC, dc, :]),
                                start=(dc == 0), stop=(dc == DC - 1))
                        nc.vector.tensor_scalar_add(
                            kT[i][:, tb * NT:(tb + 1) * NT], pk[:],
                            bqk[:, fc:fc + 1])
                    for tb in range(2):
                        pq = qps.tile([128, NT], F32, name="pq", tag="qkv")
                        for dc in range(DC):
                            nc.tensor.matmul(
                                pq[:], (w["wq"][:, dc, ws]),
                                (hT[:, tb * QBC:(tb + 1) * QBC, dc, :]),
                                start=(dc == 0), stop=(dc == DC - 1))
                        nc.vector.tensor_scalar_add(
                            qT[i][:, tb * NT:(tb + 1) * NT], pq[:],
                            bqk[:, FC + fc:FC + fc + 1])
                # --- V (token-major) ---
                vt = [vp.tile([128, FCP * 130], BF16, name=f"v{ti}",
                              tag=f"v{ti}") for ti in range(S)]
                for ti in range(S):
                    pv = qps.tile([128, FCP * 130], F32, name="pv", tag="qkv")
                    for dc in range(DC):
                        nc.tensor.matmul(
                            pv[:], (hT[:, ti, dc, :]),
                            (w["wv"][:, dc, :]),
                            start=(dc == 0), stop=(dc == DC - 1))
                    nc.vector.tensor_add(vt[ti][:], pv[:], bvb[:, p, :])
                # prefetch next pass weights (overlaps attention below)
                if p + 1 < NPASS:
                    cur_w = issue_pass_loads(p + 1)
                # --- attention per head pair ---
                for i, fc in enumerate(fcs):
                    for qb in range(2):
                        if qb == 0:
                            schunks = list(range(S))
                        else:
                            schunks = list(range(QBC, 3 * QBC))
                        ctx_ps = [cps.tile([65, NT], F32, name=f"ctx{hh}",
                                           tag=f"ctx{hh}") for hh in range(2)]
                        nsc = len(schunks)
                        for idx, sc in enumerate(schunks):
                            # mask: (kind, index); kind: 0=none,1=tri,2=scalar
                            if qb == 0:
                                if sc < QBC:
                                    mk = (1, sc)
                                elif sc >= S - QBC:
                                    mk = (2, sc - (S - QBC))
                                else:
                                    mk = (0, 0)
                            else:
                                if sc < 2 * QBC:
                                    mk = (1, sc - QBC)
                                else:
                                    mk = (2, QBC + (sc - 2 * QBC))
                            # diagonal chunks only need columns >= 128*j
                            # (floored so the moving dim stays >= 256)
                            coff = min(mk[1] * 128, NT - 256) \
                                if mk[0] == 1 else 0
                            coff = max(coff, 0)
                            ncols = NT - coff
                            sps_t = sps.tile([128, 2, NT], F32,
                                             name="sc", tag="sc")
                            e2 = ep.tile([128, 2, NT], BF16, name="e", tag="e")
                            for hh in range(2):
                                rows = slice(hh * HDf, (hh + 1) * HDf)
                                nc.tensor.matmul(
                                    sps_t[:, hh, coff:],
                                    (kT[i][rows, sc * 128:(sc + 1) * 128]),
                                    (qT[i][rows, qb * NT + coff:
                                           (qb + 1) * NT]),
                                    start=True, stop=True,
                                    tile_position=(hh * HDf, 0))
                            ebias = cms[:, mk[1]:mk[1] + 1] \
                                if mk[0] == 2 else zbias[:]
                            nc.scalar.activation(
                                e2[:, :, coff:], sps_t[:, :, coff:],
                                mybir.ActivationFunctionType.Exp,
                                bias=ebias)
                            if mk[0] == 1:
                                nc.vector.tensor_mul(
                                    e2[:, :, coff:], e2[:, :, coff:],
                                    tri[:, mk[1], coff:].unsqueeze(1)
                                    .to_broadcast([128, 2, ncols]))
                            for hh in range(2):
                                nc.tensor.matmul(
                                    ctx_ps[hh][:, coff:],
                                    (vt[sc][:, (i * 2 + hh) * 65:
                                             (i * 2 + hh) * 65 + 65]),
                                    (e2[:, hh, coff:]),
                                    start=(idx == 0), stop=(idx == nsc - 1),
                                    skip_group_check=True)
                        for hh in range(2):
                            rz = zp.tile([1, NT], BF16, name="rz", tag="rz")
                            with nc.allow_low_precision(
                                    "softmax 1/z in bf16: rel 4e-3 ok"):
                                nc.vector.reciprocal(
                                    rz[:], ctx_ps[hh][64:65, :])
                            zb = zbp.tile([64, NT], BF16, name="zb", tag="zb")
                            nc.gpsimd.partition_broadcast(zb[:], rz[:])
                            rows = slice(hh * HDf, (hh + 1) * HDf)
                            nc.vector.tensor_mul(
                                ctxT[fc][rows, qb * NT:(qb + 1) * NT],
                                ctx_ps[hh][0:64, :], zb[:])

        hT_stack.close()

        # ---------------- Phase 3: projection + fused LN2 ---------------
        h2p = top.enter_context(tc.tile_pool(name="h2Tp", bufs=1))
        x1_pool = top.enter_context(tc.tile_pool(name="x1bp", bufs=1))
        w1_pool = top.enter_context(tc.tile_pool(name="ffn_w1", bufs=1))
        w2_pool = top.enter_context(tc.tile_pool(name="ffn_w2", bufs=1))
        b1_pool = top.enter_context(tc.tile_pool(name="ffn_b1", bufs=1))
        h2T = h2p.tile([128, TOC, DC, 128], BF16, name="h2T", tag="h2T")
        x1b = [x1_pool.tile([128, Dm], F32, name=f"x1b{ti}", tag=f"x1b{ti}")
               for ti in range(TOC)]

        def issue_w1_loads(g):
            w = {}
            w1t = w1_pool.tile([128, DC, DFFm // NG], BF16,
                               name="w1g", tag="w1g")
            nc.scalar.dma_start(w1t[:], w1_d.ap()[g].rearrange(
                "dc p c -> p dc c"))
            w["w1"] = w1t
            b1t = b1_pool.tile([128, GFC], F32, name="b1t", tag="b1t")
            nc.sync.dma_start(
                b1t[:], b1_d.ap()[g * GFC:(g + 1) * GFC].rearrange(
                    "j p o -> p (j o)"))
            w["b1"] = b1t
            return w

        def issue_w2_loads(g):
            w2t = w2_pool.tile([128, GFC, Dm], BF16, name="w2", tag="w2")
            nc.sync.dma_start(w2t[:], w2_d.ap()[g].rearrange(
                "j p d -> p j d"))
            return w2t

        with tc.tile_pool(name="proj_sb", bufs=1) as prp, \
             tc.tile_pool(name="proj_x", bufs=2) as pxp, \
             tc.tile_pool(name="proj_o", bufs=2) as pop, \
             tc.tile_pool(name="ln2s", bufs=8) as lsp2, \
             tc.tile_pool(name="ln2h", bufs=2) as lph2, \
             tc.tile_pool(name="projps", bufs=4, space=bass.MemorySpace.PSUM) as pps:
            b2b = prp.tile([128, Dm], F32, name="b2b", tag="b2b")
            nc.sync.dma_start(b2b[:], b2_d.ap())
            wos = prp.tile([128, FC, Dm], BF16, name="wo", tag="wo")
            nc.scalar.dma_start(wos[:], wo_d.ap().rearrange("f p d -> p f d"))
            for ti in range(TOC):
                xo = pxp.tile([128, Dm], BF16, name="xo", tag="xo")
                nc.sync.dma_start(xo[:], xbor[ti])
                x1t = pop.tile([128, Dm], F32, name="x1t", tag="x1t")
                for oc in range(OC):
                    ppt = pps.tile([128, NO], F32, name="ppt", tag="ppt")
                    for fc in range(FC):
                        nc.tensor.matmul(
                            ppt[:],
                            (ctxT[fc][:, ti * 128:(ti + 1) * 128]),
                            (wos[:, fc, oc * NO:(oc + 1) * NO]),
                            start=(fc == 0), stop=(fc == FC - 1))
                    cols = slice(oc * NO, (oc + 1) * NO)
                    nc.vector.tensor_add(x1t[:, cols], ppt[:], xo[:, cols])
                # fused LN2 on the freshly built x1 tile
                nsub = max(1, Dm // 512)
                st6 = lsp2.tile([128, nsub, 6], F32, name="st6b", tag="st6b")
                for sb_i in range(nsub):
                    nc.vector.bn_stats(
                        st6[:, sb_i, :],
                        x1t[:, sb_i * (Dm // nsub):(sb_i + 1) * (Dm // nsub)])
                agg = lsp2.tile([128, 2], F32, name="aggb", tag="aggb")
                nc.vector.bn_aggr(agg[:], st6[:])
                veps = lsp2.tile([128, 1], F32, name="vepsb", tag="vepsb")
                nc.vector.tensor_scalar_add(veps[:], agg[:, 1:2], EPS)
                std = lsp2.tile([128, 1], F32, name="stdb", tag="stdb")
                nc.scalar.sqrt(std[:], veps[:])
                rstd = lsp2.tile([128, 1], F32, name="rstdb", tag="rstdb")
                nc.vector.reciprocal(rstd[:], std[:])
                hb = lph2.tile([128, Dm], BF16, name="hb", tag="hb")
                hstep = Dm // nsub
                for sb_i in range(nsub):
                    cs = slice(sb_i * hstep, (sb_i + 1) * hstep)
                    eng = nc.vector if sb_i % 2 == 0 else nc.gpsimd
                    eng.tensor_scalar(hb[:, cs], x1t[:, cs],
                                      agg[:, 0:1], rstd[:],
                                      op0=mybir.AluOpType.subtract,
                                      op1=mybir.AluOpType.mult)
                # x1b = x1 + b2 (pre-folded residual+bias for the FFN tail)
                nc.gpsimd.tensor_add(x1b[ti][:], x1t[:], b2b[:])
                eng = nc.sync if ti % 2 == 0 else nc.scalar
                eng.dma_start_transpose(h2T[:, ti, :, :], hb[:])
            ffn_w = issue_w1_loads(0)

        # ---------------- Phase 4: FFN ----------------------------------
        with tc.tile_pool(name="ffn_ff", bufs=1) as fp, \
             tc.tile_pool(name="ffn_out", bufs=3) as fop, \
             tc.tile_pool(name="ffps", bufs=3, space=bass.MemorySpace.PSUM) as fps, \
             tc.tile_pool(name="outps", bufs=3, space=bass.MemorySpace.PSUM) as ops:
            ffT = [fp.tile([128, TOWN], BF16, name=f"ffT{j}", tag=f"ffT{j}")
                   for j in range(GFC)]
            for g in range(NG):
                w = ffn_w
                w2s = issue_w2_loads(g)
                for j in range(GFC):
                    for tb in range(TOWN // NT):
                        fpt = fps.tile([128, NT], F32, name="fpt", tag="fpt")
                        for dc in range(DC):
                            nc.tensor.matmul(
                                fpt[:], (w["w1"][:, dc, j * 128:(j + 1) * 128]),
                                (h2T[:, tb * QBC:(tb + 1) * QBC, dc, :]),
                                start=(dc == 0), stop=(dc == DC - 1))
                        nc.scalar.activation(
                            ffT[j][:, tb * NT:(tb + 1) * NT], fpt[:],
                            mybir.ActivationFunctionType.Relu,
                            bias=w["b1"][:, j:j + 1])
                if g + 1 < NG:
                    ffn_w = issue_w1_loads(g + 1)
                for ti in range(TOC):
                    for oc in range(OC):
                        opt = ops.tile([128, NO], F32, name="opt", tag="opt")
                        for j in range(GFC):
                            nc.tensor.matmul(
                                opt[:],
                                (ffT[j][:, ti * 128:(ti + 1) * 128]),
                                (w2s[:, j, oc * NO:(oc + 1) * NO]),
                                start=(j == 0), stop=(j == GFC - 1))
                        cols = slice(oc * NO, (oc + 1) * NO)
                        if g < NG - 1:
                            nc.vector.tensor_add(x1b[ti][:, cols],
                                                 x1b[ti][:, cols], opt[:])
                        else:
                            ot = fop.tile([128, NO], F32, name="ot", tag="ot")
                            nc.vector.tensor_add(ot[:], x1b[ti][:, cols],
                                                 opt[:])
                            nc.sync.dma_start(outr[ti][:, cols], ot[:])
    nc.compile()
    return nc


# ---------------------------------------------------------------------------
# host-side input preparation
# ---------------------------------------------------------------------------

def _bf16(a):
    return np.ascontiguousarray(np.asarray(a, np.float32)
                                .astype(ml_dtypes.bfloat16))


def prepare_shared(cfg, Wq, Wk, Wv, Wo, bo, W1, b1, W2, b2, g1, be1, g2, be2):
    c = derive(cfg)
    Dm, Hn, DFFm, FC, GFC = cfg["D"], cfg["H"], cfg["DFF"], c["FC"], c["GFC"]
    scale = 1.0 / np.sqrt(HD)
    wq_f = np.ascontiguousarray(Wq.transpose(1, 0, 2).reshape(Dm, Hn * HD))
    wk_f = np.ascontiguousarray(Wk.transpose(1, 0, 2).reshape(Dm, Hn * HD))
    wv_f = np.ascontiguousarray(Wv.transpose(1, 0, 2).reshape(Dm, Hn * HD))
    wq_e = (g1[:, None] * wq_f) * scale
    wk_e = g1[:, None] * wk_f
    wv_e = g1[:, None] * wv_f
    bq = ((be1 @ wq_f) * scale).reshape(FC, 128, 1)
    bk = (be1 @ wk_f).reshape(FC, 128, 1)
    bv = (be1 @ wv_f).reshape(1, Hn * HD)
    w1_e = g2[:, None] * W1
    b1_e = (b1 + be2 @ W1).reshape(DFFm // 128, 128, 1)
    DC, NPASS, FCP, NG = c["DC"], c["NPASS"], c["FCP"], cfg["NG"]

    def qkv_tile(w):
        # [D, F] -> [NPASS, DC, 128, FCP*128]
        return w.reshape(DC, 128, NPASS, FCP * 128).transpose(2, 0, 1, 3)

    # v weights get a zero column appended per head; its bias is 1.0, so the
    # v tiles come out of the matmul+bias with a built-in ones column that
    # accumulates the softmax normalizer during the ctx matmul.
    nheads = FCP * 2
    wv_r = wv_e.reshape(DC, 128, NPASS, nheads, HD)
    wv_a = np.concatenate(
        [wv_r, np.zeros((DC, 128, NPASS, nheads, 1), wv_r.dtype)], axis=-1)
    wv_t = wv_a.transpose(2, 0, 1, 3, 4).reshape(NPASS, DC, 128, nheads * 65)
    bv_r = bv.reshape(NPASS, nheads, HD)
    bv_a = np.concatenate(
        [bv_r, np.ones((NPASS, nheads, 1), bv_r.dtype)], axis=-1)
    bv_t = bv_a.reshape(NPASS, 1, nheads * 65)

    w1_t = w1_e.reshape(DC, 128, NG, DFFm // NG).transpose(2, 0, 1, 3)
    w2_t = W2.reshape(NG, GFC, 128, Dm)
    f32c = lambda a: np.ascontiguousarray(a, dtype=np.float32)
    return dict(
        wq=_bf16(qkv_tile(wq_e)), wk=_bf16(qkv_tile(wk_e)),
        wv=_bf16(wv_t), bv=f32c(bv_t),
        bq=f32c(bq), bk=f32c(bk),
        wo=_bf16(Wo.reshape(FC, 128, Dm)),
        w1=_bf16(w1_t), b1=f32c(b1_e),
        w2=_bf16(w2_t),
        b2=f32c(np.broadcast_to(b2.reshape(1, Dm), (128, Dm))),
        zeros=np.zeros((128, 1), np.float32),
        bo=f32c(bo),
    )


def core_plan(cfg, half):
    """Return (perm, qposA, qposB) token index arrays for one core."""
    QB = cfg["QB"]
    Tf = cfg["T"]
    nb = Tf // QB  # 4 blocks
    if half == 0:
        bA, bB = nb - 1, 0
    else:
        bA, bB = nb - 2, 1
    own = {bA, bB}
    restA = [b for b in range(nb) if b not in own and b < bA]
    restB = [b for b in range(nb) if b not in own and b >= bA]
    blocks = [bA, bB] + restA + restB
    perm = np.concatenate([np.arange(b * QB, (b + 1) * QB) for b in blocks])
    qposA = np.arange(bA * QB, (bA + 1) * QB)
    qposB = np.arange(bB * QB, (bB + 1) * QB)
    return perm, qposA, qposB


def make_masks(cfg, perm, qposA, qposB):
    """tri tiles [QBC,128,NT]; whole-chunk exp-bias scalars (0 / -80)."""
    c = derive(cfg)
    QBC, NT, S = c["QBC"], c["NT"], c["S"]
    key = perm
    tri = np.zeros((QBC, 128, NT), np.float32)
    for j in range(QBC):
        ks = key[j * 128:(j + 1) * 128]
        tri[j] = (ks[:, None] <= qposA[None, :]).astype(np.float32)
    cm = np.zeros((2 * QBC, 128, 1), np.float32)
    for j in range(QBC):
        sc = S - QBC + j
        ks = key[sc * 128:(sc + 1) * 128]
        m = ks[:, None] <= qposA[None, :]
        assert m.all() or not m.any(), "chunk not homogeneous"
        cm[j] = 0.0 if m.all() else -80.0
    for j in range(QBC):
        sc = 2 * QBC + j
        ks = key[sc * 128:(sc + 1) * 128]
        m = ks[:, None] <= qposB[None, :]
        assert m.all() or not m.any(), "chunk not homogeneous"
        cm[QBC + j] = 0.0 if m.all() else -80.0
    return tri, cm


def core_inputs(cfg, shared, x, core):
    """Build the per-core input map. Returns (in_map, (batch, perm))."""
    c = derive(cfg)
    b, half = core // 2, core % 2
    perm, qposA, qposB = core_plan(cfg, half)
    tri, cm = make_masks(cfg, perm, qposA, qposB)
    m = dict(shared)
    bo = m.pop("bo")
    xp = np.asarray(x[b][perm], np.float32)
    m["x"] = _bf16(xp)
    m["xbo"] = _bf16(xp[:c["TOWN"]] + bo[None, :])
    m["tri"] = _bf16(tri)
    m["cm"] = np.ascontiguousarray(cm, np.float32)
    return m, (b, perm)


_NC_CACHE = {}

# test-harness knobs (ignored in normal grading use)
TRACE = False
TRACE_KWARGS = {}
LAST_RESULT = None


def _get_nc(key, cfg):
    if key not in _NC_CACHE:
        _NC_CACHE[key] = build(cfg)
    return _NC_CACHE[key]


def kernel(x, Wq, Wk, Wv, Wo, bo, W1, b1, W2, b2, g1, be1, g2, be2):
    cfg = FULL_CFG
    c = derive(cfg)
    x = np.asarray(x, np.float32)
    shared = prepare_shared(cfg, np.asarray(Wq), np.asarray(Wk), np.asarray(Wv),
                            np.asarray(Wo), np.asarray(bo), np.asarray(W1),
                            np.asarray(b1), np.asarray(W2), np.asarray(b2),
                            np.asarray(g1), np.asarray(be1), np.asarray(g2),
                            np.asarray(be2))
    nc = _get_nc("full", cfg)
    in_maps = []
    plans = []
    for core in range(N_CORES):
        m, plan = core_inputs(cfg, shared, x, core)
        in_maps.append(m)
        plans.append(plan)
    res = run_bass_kernel_spmd(nc, in_maps, list(range(N_CORES)),
                               trace=TRACE, **TRACE_KWARGS)
    global LAST_RESULT
    LAST_RESULT = res
    out = np.zeros((B, T, D), np.float32)
    TOWN = c["TOWN"]
    for core in range(N_CORES):
        b, perm = plans[core]
        o = res.results[core]["out"]
        out[b][perm[:TOWN]] = o
    return out
